# revision 19
# baseline (speedup 1.0000x reference)
"""Trainium2 kernel for nn_HadamardRotation: y = x @ H, H = 4096x4096 Walsh-Hadamard.

Strategy
--------
H4096 = H64 (x) H64 (Kronecker). Writing d = 64*hi + lo, e = 64*hi' + lo':

    y[r, e] = sum_{hi,lo} H64[lo,lo'] * H64[hi,hi'] * x[r, d]

Two matmul stages with 128-wide contraction (block-diagonal I2 (x) H64
weights), separated by an on-chip "corner turn" (SBUF->SBUF DMA partition
shuffle), all operating in the transposed domain (d on partitions, rows on
the free axis). Host does the cheap transposes / index unscrambles; the
device only ever issues contiguous 4KB DMA lines.

FLOPs: 2 * 128/4096 of the naive matmul = 16x reduction.

Data parallel over 8 cores: rows sharded 16384 -> 8 x 2048, weights
replicated.

Layouts (per core, R = 2048 rows):
  xt  DRAM in  (32, 128, R): xt[a, 64*mu+lo, r] = x[r, 128*a + 64*mu + lo]
  B1  (128,128): B1[64*mu+lo, 2*lo'+mu]     = H64[lo, lo']
  B2  (128,128): B2[64*nu+32*mu+a, 2*hi'+nu] = H64[2*a+mu, hi']
  stage A (chunk a): u[p, a, r] = sum_k B1[k, p] xt[a, k, r]
      => u[4c + (2*nu+mu), a] holds (hi = 2a+mu, lo' = 2c+nu)
  corner turn:  v_c[32*t + a, r] = u[4*c + t, a, r]
  stage B (chunk c): Y[c, m, r] = sum_q B2[q, m] v_c[q, r]
      => Y[c, 2*hi'+nu, r] = y[r, 64*hi' + 2*c + nu]

Perf notes (from NTFF profiles): the kernel is DMA-bound. All bulk DMA
(input load, corner turn, output store) is issued from the SP engine so it
lands on qSyncDynamicHW, the only queue striped over all 16 DMA engines
(the scalar/ACT HWDGE queue only gets 4). Every transfer moves 4KB
contiguous lines. Output is written bf16 (host upcasts) to halve the HBM
write. Total DMA = 3 x 16.8MB per core ~= 50MB at 360 GB/s aggregate.
"""

import math
import numpy as np
import ml_dtypes

import concourse.bass as bass
import concourse.mybir as mybir
import concourse.tile as tile
from concourse import bacc
from concourse.bass_utils import run_bass_kernel_spmd

N_CORES = 8
DIM = 4096
R_TOTAL = 4 * 4096          # rows after flattening (4, 4096, DIM)
R = R_TOTAL // N_CORES      # rows per core
N = 512                     # matmul free-dim slab (one PSUM bank of fp32)
NSLICE = R // N             # matmul slices per chunk

MODE = "bf16"

CFG = dict(
    in_eng="sync",
    hop1_eng="sync",
    hop2_eng="sync",
    out_eng="sync",
    copy_rot=("vector", "any"),  # engines for psum->sbuf copies, round robin
    xbufs=3, ubufs=3, vbufs=3, ybufs=3,
    in_b=2,                      # input chunks per DMA (also hop-1 batch)
    out_b=2,                     # output chunks per DMA
    turn_g=4,                    # c-chunks per hop-2 gather
)


def _walsh_hadamard64():
    h = np.array([[1.0]], dtype=np.float64)
    while h.shape[0] < 64:
        h = np.block([[h, h], [h, -h]]) / math.sqrt(2.0)
    return h.astype(np.float32)


def _build_weights(H64):
    B1 = np.zeros((128, 128), dtype=np.float32)
    b1v = B1.reshape(2, 64, 64, 2)
    for mu in range(2):
        b1v[mu, :, :, mu] = H64
    B2 = np.zeros((128, 128), dtype=np.float32)
    b2v = B2.reshape(2, 2, 32, 64, 2)
    for nu in range(2):
        for mu in range(2):
            b2v[nu, mu, :, :, nu] = H64[mu::2, :]
    return B1, B2


_NC_CACHE = {}


def _build_bass(mode, loop=0, cfg=None):
    cfg = dict(CFG, **(cfg or {}))
    key = (mode, loop, tuple(sorted((k, str(v)) for k, v in cfg.items())))
    if key in _NC_CACHE:
        return _NC_CACHE[key]

    f32 = mybir.dt.float32
    dt_in = mybir.dt.bfloat16 if mode == "bf16" else f32
    dt_out = mybir.dt.bfloat16 if mode == "bf16" else f32

    nc = bacc.Bacc("TRN2", target_bir_lowering=False, debug=False,
                   num_devices=N_CORES)
    xt_d = nc.dram_tensor("xt", [32, 128, R], dt_in, kind="ExternalInput")
    B1_d = nc.dram_tensor("B1", [128, 128], dt_in, kind="ExternalInput")
    B2_d = nc.dram_tensor("B2", [128, 128], dt_in, kind="ExternalInput")
    Y_d = nc.dram_tensor("Y", [32, 128, R], dt_out, kind="ExternalOutput")
    # corner-turn scratch, layout [c, t, a, r]: T[c,t,a,r] = u[4c+t, a, r].
    # Routing the turn through DRAM keeps every DMA a clean 128-partition
    # HWDGE transfer (stripes over all 16 DMA engines with 4KB lines);
    # direct SBUF->SBUF turns are limited to 4 engines (HWDGE, one
    # descriptor chain per source partition) or ~6.3 GB/s/engine (SWDGE).
    T_d = nc.dram_tensor("Tsc", [32, 4, 32, R], dt_in, kind="Internal")

    with tile.TileContext(nc) as tc:
        with (
            tc.tile_pool(name="wpool", bufs=1) as wpool,
            tc.tile_pool(name="xpool", bufs=cfg["xbufs"]) as xpool,
            tc.tile_pool(name="upool", bufs=cfg["ubufs"]) as upool,
            tc.tile_pool(name="vpool", bufs=cfg["vbufs"]) as vpool,
            tc.tile_pool(name="ypool", bufs=cfg["ybufs"]) as ypool,
            tc.tile_pool(name="psA", bufs=4, space="PSUM") as psA,
            tc.tile_pool(name="psB", bufs=4, space="PSUM") as psB,
        ):
            B1_sb = wpool.tile([128, 128], dt_in)
            nc.sync.dma_start(B1_sb[:], B1_d[:])
            B2_sb = wpool.tile([128, 128], dt_in)
            nc.sync.dma_start(B2_sb[:], B2_d[:])

            in_eng = getattr(nc, cfg["in_eng"])
            hop1_eng = getattr(nc, cfg["hop1_eng"])
            hop2_eng = getattr(nc, cfg["hop2_eng"])
            out_eng = getattr(nc, cfg["out_eng"])
            rot = cfg["copy_rot"]

            def copy(dst, src, i):
                getattr(nc, rot[i % len(rot)]).tensor_copy(dst, src)

            IB = cfg["in_b"]
            OB = cfg["out_b"]
            TG = cfg["turn_g"]

            def body():
                # ---- phase A: load chunks, stage-A matmuls, spill u to T ----
                ci = 0
                hop_pend = []
                for ab in range(32 // IB):
                    xg = xpool.tile([128, IB, R], dt_in)
                    in_eng.dma_start(
                        xg[:], xt_d[IB * ab:IB * (ab + 1)].transpose([1, 0, 2]))
                    # delay hop1 issue one batch so it doesn't head-of-line
                    # block the next input load on the SP queue
                    while hop_pend:
                        ab_, ua_ = hop_pend.pop(0)
                        hop1_eng.dma_start(
                            T_d[:, :, IB * ab_:IB * (ab_ + 1), :], ua_[:])
                    ua = upool.tile([128, IB, R], dt_in)
                    for j in range(IB):
                        a = IB * ab + j
                        for s in range(NSLICE):
                            pu = psA.tile([128, N], f32)
                            nc.tensor.matmul(pu[:], B1_sb[:],
                                             xg[:, j, s * N:(s + 1) * N],
                                             start=True, stop=True)
                            copy(ua[:, j, s * N:(s + 1) * N], pu[:], ci)
                            ci += 1
                    hop_pend.append((ab, ua))
                for ab_, ua_ in hop_pend:
                    hop1_eng.dma_start(
                        T_d[:, :, IB * ab_:IB * (ab_ + 1), :], ua_[:])

                # ---- phase B: gather v from T, stage-B matmuls, store ----
                def hop2(g):
                    # v[32t+a, c', r] = T[TG*g+c', t, a, r]; (t, a) flat on
                    # the DRAM side with stride R, so 3-dim gather.
                    vg = vpool.tile([128, TG, R], dt_in)
                    src = bass.AP(T_d, TG * g * 128 * R,
                                  [[R, 128], [128 * R, TG], [1, R]])
                    hop2_eng.dma_start(vg[:], src)
                    return vg

                def stageB(g, vg, ci):
                    for cb in range(TG // OB):
                        yb = ypool.tile([128, OB, R], dt_out)
                        for j in range(OB):
                            cj = cb * OB + j
                            for s in range(NSLICE):
                                py = psB.tile([128, N], f32)
                                nc.tensor.matmul(py[:], B2_sb[:],
                                                 vg[:, cj, s * N:(s + 1) * N],
                                                 start=True, stop=True)
                                copy(yb[:, j, s * N:(s + 1) * N], py[:], ci)
                                ci += 1
                        c0 = TG * g + cb * OB
                        out_eng.dma_start(
                            Y_d[c0:c0 + OB].transpose([1, 0, 2]), yb[:])
                    return ci

                pend = None
                for g in range(32 // TG):
                    vg = hop2(g)
                    if pend is not None:
                        ci = stageB(*pend, ci)
                    pend = (g, vg)
                ci = stageB(*pend, ci)

            def body_sched(sched):
                # like body() but with an explicit hop-2 group size schedule
                # (e.g. [16, 8, 4, 2, 2]): large groups early amortize
                # per-group pipeline stalls, small groups at the end shorten
                # the matmul+copy drain after the last gather.
                ci = 0
                hop_pend = []
                for ab in range(32 // IB):
                    xg = xpool.tile([128, IB, R], dt_in)
                    in_eng.dma_start(
                        xg[:], xt_d[IB * ab:IB * (ab + 1)].transpose([1, 0, 2]))
                    while hop_pend:
                        ab_, ua_ = hop_pend.pop(0)
                        hop1_eng.dma_start(
                            T_d[:, :, IB * ab_:IB * (ab_ + 1), :], ua_[:])
                    ua = upool.tile([128, IB, R], dt_in)
                    for j in range(IB):
                        for s in range(NSLICE):
                            pu = psA.tile([128, N], f32)
                            nc.tensor.matmul(pu[:], B1_sb[:],
                                             xg[:, j, s * N:(s + 1) * N],
                                             start=True, stop=True)
                            copy(ua[:, j, s * N:(s + 1) * N], pu[:], ci)
                            ci += 1
                    hop_pend.append((ab, ua))
                for ab_, ua_ in hop_pend:
                    hop1_eng.dma_start(
                        T_d[:, :, IB * ab_:IB * (ab_ + 1), :], ua_[:])

                def hop2(c0, tg):
                    vg = vpool.tile([128, tg, R], dt_in, name=f"v{tg}")
                    src = bass.AP(T_d, c0 * 128 * R,
                                  [[R, 128], [128 * R, tg], [1, R]])
                    hop2_eng.dma_start(vg[:], src)
                    return vg

                def stageB(c0, tg, vg, ci):
                    for cb in range(tg // OB):
                        yb = ypool.tile([128, OB, R], dt_out)
                        for j in range(OB):
                            cj = cb * OB + j
                            for s in range(NSLICE):
                                py = psB.tile([128, N], f32)
                                nc.tensor.matmul(py[:], B2_sb[:],
                                                 vg[:, cj, s * N:(s + 1) * N],
                                                 start=True, stop=True)
                                copy(yb[:, j, s * N:(s + 1) * N], py[:], ci)
                                ci += 1
                        cc = c0 + cb * OB
                        out_eng.dma_start(
                            Y_d[cc:cc + OB].transpose([1, 0, 2]), yb[:])
                    return ci

                assert sum(sched) == 32
                c0 = 0
                pend = None
                for tg in sched:
                    vg = hop2(c0, tg)
                    if pend is not None:
                        ci = stageB(*pend, ci)
                    pend = (c0, tg, vg)
                    c0 += tg
                ci = stageB(*pend, ci)

            sched = cfg.get("sched")
            run = (lambda: body_sched(sched)) if sched else body
            if loop:
                with tc.For_i(0, loop, 1):
                    run()
            else:
                run()

    nc.compile()
    _NC_CACHE[key] = nc
    return nc


def _prep_inputs(x, H, mode):
    np_in = ml_dtypes.bfloat16 if mode == "bf16" else np.float32
    H64 = (np.asarray(H, dtype=np.float32)[::64, ::64] * 8.0).astype(np.float32)
    B1, B2 = _build_weights(H64)
    B1 = B1.astype(np_in)
    B2 = B2.astype(np_in)
    xf = np.asarray(x, dtype=np.float32).reshape(R_TOTAL, DIM)
    in_maps = []
    for i in range(N_CORES):
        shard = xf[i * R:(i + 1) * R]                     # (R, DIM)
        xt = np.ascontiguousarray(shard.T, dtype=np_in)   # (DIM, R)
        xt = xt.reshape(32, 128, R)
        in_maps.append({"xt": xt, "B1": B1, "B2": B2})
    return in_maps


def _unscramble(results):
    outs = []
    for i in range(N_CORES):
        Y = results[i]["Y"]                               # (32, 128, R)
        y = Y.reshape(32, 64, 2, R).transpose(3, 1, 0, 2).reshape(R, DIM)
        outs.append(y)
    return np.concatenate(outs, axis=0).reshape(4, 4096, DIM).astype(np.float32)


def _install_ntff_hook():
    """Provide antenv.axon_hooks.get_axon_ntff_profile_hook via ctypes on
    the baked libaxon_pjrt.so (the agent image lacks the module). Only used
    on the _trace path."""
    import sys, types, ctypes, contextlib
    if "antenv.axon_hooks" in sys.modules:
        return
    try:
        lib = ctypes.CDLL("/opt/axon/libaxon_pjrt.so")
        if not hasattr(lib, "axon_start_nrt_profile"):
            return
    except OSError:
        return
    lib.axon_start_nrt_profile.argtypes = [ctypes.POINTER(ctypes.c_int64),
                                           ctypes.c_size_t]
    lib.axon_start_nrt_profile.restype = ctypes.c_int64
    lib.axon_stop_nrt_profile.argtypes = [ctypes.c_char_p]
    lib.axon_stop_nrt_profile.restype = ctypes.c_int64

    @contextlib.contextmanager
    def _hook(output_dir, device_ids):
        import jax
        jax.devices()
        if device_ids:
            ids = (ctypes.c_int64 * len(device_ids))(*device_ids)
            rc = lib.axon_start_nrt_profile(ids, len(device_ids))
        else:
            rc = lib.axon_start_nrt_profile(None, 0)
        if rc != 0:
            raise RuntimeError(f"axon_start_nrt_profile rc={rc}")
        try:
            yield
        finally:
            n = lib.axon_stop_nrt_profile(str(output_dir).encode())
            print(f"ntff profile: {n} file(s) -> {output_dir}")

    mod = types.ModuleType("antenv.axon_hooks")
    mod.get_axon_ntff_profile_hook = lambda: _hook
    sys.modules["antenv.axon_hooks"] = mod


def kernel(x, H, _trace=False, _loop=0, _cfg=None):
    if _trace:
        _install_ntff_hook()
        from concourse import bass_utils as _bu
        _bu.upload_artifacts = lambda d: d
    nc = _build_bass(MODE, loop=_loop, cfg=_cfg)
    in_maps = _prep_inputs(x, H, MODE)
    res = run_bass_kernel_spmd(nc, in_maps, core_ids=list(range(N_CORES)),
                               trace=_trace)
    out = _unscramble(res.results)
    if _trace:
        return out, res
    return out


# revision 20
# speedup vs baseline: 1.1145x; 1.1145x over previous
"""Trainium2 kernel for nn_HadamardRotation: y = x @ H, H = 4096x4096 Walsh-Hadamard.

Strategy
--------
H4096 = H64 (x) H64 (Kronecker). Writing d = 64*hi + lo, e = 64*hi' + lo':

    y[r, e] = sum_{hi,lo} H64[lo,lo'] * H64[hi,hi'] * x[r, d]

Two matmul stages with 128-wide contraction (block-diagonal I2 (x) H64
weights), separated by an on-chip "corner turn" (SBUF->SBUF DMA partition
shuffle), all operating in the transposed domain (d on partitions, rows on
the free axis). Host does the cheap transposes / index unscrambles; the
device only ever issues contiguous 4KB DMA lines.

FLOPs: 2 * 128/4096 of the naive matmul = 16x reduction.

Data parallel over 8 cores: rows sharded 16384 -> 8 x 2048, weights
replicated.

Layouts (per core, R = 2048 rows):
  xt  DRAM in  (32, 128, R): xt[a, 64*mu+lo, r] = x[r, 128*a + 64*mu + lo]
  B1  (128,128): B1[64*mu+lo, 2*lo'+mu]     = H64[lo, lo']
  B2  (128,128): B2[64*nu+32*mu+a, 2*hi'+nu] = H64[2*a+mu, hi']
  stage A (chunk a): u[p, a, r] = sum_k B1[k, p] xt[a, k, r]
      => u[4c + (2*nu+mu), a] holds (hi = 2a+mu, lo' = 2c+nu)
  corner turn:  v_c[32*t + a, r] = u[4*c + t, a, r]
  stage B (chunk c): Y[c, m, r] = sum_q B2[q, m] v_c[q, r]
      => Y[c, 2*hi'+nu, r] = y[r, 64*hi' + 2*c + nu]

Perf notes (from NTFF profiles): the kernel is DMA-bound. All bulk DMA
(input load, corner turn, output store) is issued from the SP engine so it
lands on qSyncDynamicHW, the only queue striped over all 16 DMA engines
(the scalar/ACT HWDGE queue only gets 4). Every transfer moves 4KB
contiguous lines. Output is written bf16 (host upcasts) to halve the HBM
write. Total DMA = 3 x 16.8MB per core ~= 50MB at 360 GB/s aggregate.
"""

import math
import numpy as np
import ml_dtypes

import concourse.bass as bass
import concourse.mybir as mybir
import concourse.tile as tile
from concourse import bacc
from concourse.bass_utils import run_bass_kernel_spmd

N_CORES = 8
DIM = 4096
R_TOTAL = 4 * 4096          # rows after flattening (4, 4096, DIM)
R = R_TOTAL // N_CORES      # rows per core
N = 512                     # matmul free-dim slab (one PSUM bank of fp32)
NSLICE = R // N             # matmul slices per chunk

MODE = "bf16"

CFG = dict(
    in_eng="sync",
    hop1_eng="sync",
    hop2_eng="sync",
    out_eng="sync",
    copy_rot=("vector", "any"),  # engines for psum->sbuf copies, round robin
    xbufs=3, ubufs=3, vbufs=3, ybufs=3,
    in_b=2,                      # input chunks per DMA (also hop-1 batch)
    out_b=2,                     # output chunks per DMA
    turn_g=4,                    # c-chunks per hop-2 gather
)


def _walsh_hadamard64():
    h = np.array([[1.0]], dtype=np.float64)
    while h.shape[0] < 64:
        h = np.block([[h, h], [h, -h]]) / math.sqrt(2.0)
    return h.astype(np.float32)


def _build_weights(H64):
    B1 = np.zeros((128, 128), dtype=np.float32)
    b1v = B1.reshape(2, 64, 64, 2)
    for mu in range(2):
        b1v[mu, :, :, mu] = H64
    B2 = np.zeros((128, 128), dtype=np.float32)
    b2v = B2.reshape(2, 2, 32, 64, 2)
    for nu in range(2):
        for mu in range(2):
            b2v[nu, mu, :, :, nu] = H64[mu::2, :]
    return B1, B2


_NC_CACHE = {}


def _build_bass(mode, loop=0, cfg=None):
    cfg = dict(CFG, **(cfg or {}))
    key = (mode, loop, tuple(sorted((k, str(v)) for k, v in cfg.items())))
    if key in _NC_CACHE:
        return _NC_CACHE[key]

    f32 = mybir.dt.float32
    dt_in = mybir.dt.bfloat16 if mode == "bf16" else f32
    dt_out = mybir.dt.bfloat16 if mode == "bf16" else f32

    nc = bacc.Bacc("TRN2", target_bir_lowering=False, debug=False,
                   num_devices=N_CORES)
    xt_d = nc.dram_tensor("xt", [32, 128, R], dt_in, kind="ExternalInput")
    B1_d = nc.dram_tensor("B1", [128, 128], dt_in, kind="ExternalInput")
    B2_d = nc.dram_tensor("B2", [128, 128], dt_in, kind="ExternalInput")
    Y_d = nc.dram_tensor("Y", [32, 128, R], dt_out, kind="ExternalOutput")
    # corner-turn scratch, layout [c, t, a, r]: T[c,t,a,r] = u[4c+t, a, r].
    # Routing the turn through DRAM keeps every DMA a clean 128-partition
    # HWDGE transfer (stripes over all 16 DMA engines with 4KB lines);
    # direct SBUF->SBUF turns are limited to 4 engines (HWDGE, one
    # descriptor chain per source partition) or ~6.3 GB/s/engine (SWDGE).
    T_d = nc.dram_tensor("Tsc", [32, 4, 32, R], dt_in, kind="Internal")

    with tile.TileContext(nc) as tc:
        with (
            tc.tile_pool(name="wpool", bufs=1) as wpool,
            tc.tile_pool(name="xpool", bufs=cfg["xbufs"]) as xpool,
            tc.tile_pool(name="upool", bufs=cfg["ubufs"]) as upool,
            tc.tile_pool(name="vpool", bufs=cfg["vbufs"]) as vpool,
            tc.tile_pool(name="ypool", bufs=cfg["ybufs"]) as ypool,
            tc.tile_pool(name="psA", bufs=4, space="PSUM") as psA,
            tc.tile_pool(name="psB", bufs=4, space="PSUM") as psB,
        ):
            B1_sb = wpool.tile([128, 128], dt_in)
            nc.sync.dma_start(B1_sb[:], B1_d[:])
            B2_sb = wpool.tile([128, 128], dt_in)
            nc.sync.dma_start(B2_sb[:], B2_d[:])

            in_eng = getattr(nc, cfg["in_eng"])
            hop1_eng = getattr(nc, cfg["hop1_eng"])
            hop2_eng = getattr(nc, cfg["hop2_eng"])
            out_eng = getattr(nc, cfg["out_eng"])
            rot = cfg["copy_rot"]

            def copy(dst, src, i):
                getattr(nc, rot[i % len(rot)]).tensor_copy(dst, src)

            IB = cfg["in_b"]
            OB = cfg["out_b"]
            TG = cfg["turn_g"]

            def body():
                # ---- phase A: load chunks, stage-A matmuls, spill u to T ----
                ci = 0
                hop_pend = []
                for ab in range(32 // IB):
                    xg = xpool.tile([128, IB, R], dt_in)
                    in_eng.dma_start(
                        xg[:], xt_d[IB * ab:IB * (ab + 1)].transpose([1, 0, 2]))
                    # delay hop1 issue one batch so it doesn't head-of-line
                    # block the next input load on the SP queue
                    while hop_pend:
                        ab_, ua_ = hop_pend.pop(0)
                        hop1_eng.dma_start(
                            T_d[:, :, IB * ab_:IB * (ab_ + 1), :], ua_[:])
                    ua = upool.tile([128, IB, R], dt_in)
                    for j in range(IB):
                        a = IB * ab + j
                        for s in range(NSLICE):
                            pu = psA.tile([128, N], f32)
                            nc.tensor.matmul(pu[:], B1_sb[:],
                                             xg[:, j, s * N:(s + 1) * N],
                                             start=True, stop=True)
                            copy(ua[:, j, s * N:(s + 1) * N], pu[:], ci)
                            ci += 1
                    hop_pend.append((ab, ua))
                for ab_, ua_ in hop_pend:
                    hop1_eng.dma_start(
                        T_d[:, :, IB * ab_:IB * (ab_ + 1), :], ua_[:])

                # ---- phase B: gather v from T, stage-B matmuls, store ----
                HS = cfg.get("hop_sub") or TG

                def hop2(g):
                    # v[32t+a, c', r] = T[TG*g+c', t, a, r]; (t, a) flat on
                    # the DRAM side with stride R, so 3-dim gather. Split
                    # into HS-chunk sub-DMAs writing disjoint vg sub-ranges:
                    # subtile dep tracking lets stage-B matmuls chase each
                    # sub-gather instead of waiting for the whole group.
                    vg = vpool.tile([128, TG, R], dt_in)
                    for cs in range(TG // HS):
                        src = bass.AP(T_d, (TG * g + HS * cs) * 128 * R,
                                      [[R, 128], [128 * R, HS], [1, R]])
                        hop2_eng.dma_start(vg[:, HS * cs:HS * (cs + 1), :], src)
                    return vg

                def stageB(g, vg, ci):
                    for cb in range(TG // OB):
                        yb = ypool.tile([128, OB, R], dt_out)
                        for j in range(OB):
                            cj = cb * OB + j
                            for s in range(NSLICE):
                                py = psB.tile([128, N], f32)
                                nc.tensor.matmul(py[:], B2_sb[:],
                                                 vg[:, cj, s * N:(s + 1) * N],
                                                 start=True, stop=True)
                                copy(yb[:, j, s * N:(s + 1) * N], py[:], ci)
                                ci += 1
                        c0 = TG * g + cb * OB
                        out_eng.dma_start(
                            Y_d[c0:c0 + OB].transpose([1, 0, 2]), yb[:])
                    return ci

                pend = None
                for g in range(32 // TG):
                    vg = hop2(g)
                    if pend is not None:
                        ci = stageB(*pend, ci)
                    pend = (g, vg)
                ci = stageB(*pend, ci)

            def body_sched(sched):
                # like body() but with an explicit hop-2 group size schedule
                # (e.g. [16, 8, 4, 2, 2]): large groups early amortize
                # per-group pipeline stalls, small groups at the end shorten
                # the matmul+copy drain after the last gather.
                ci = 0
                hop_pend = []
                for ab in range(32 // IB):
                    xg = xpool.tile([128, IB, R], dt_in)
                    in_eng.dma_start(
                        xg[:], xt_d[IB * ab:IB * (ab + 1)].transpose([1, 0, 2]))
                    while hop_pend:
                        ab_, ua_ = hop_pend.pop(0)
                        hop1_eng.dma_start(
                            T_d[:, :, IB * ab_:IB * (ab_ + 1), :], ua_[:])
                    ua = upool.tile([128, IB, R], dt_in)
                    for j in range(IB):
                        for s in range(NSLICE):
                            pu = psA.tile([128, N], f32)
                            nc.tensor.matmul(pu[:], B1_sb[:],
                                             xg[:, j, s * N:(s + 1) * N],
                                             start=True, stop=True)
                            copy(ua[:, j, s * N:(s + 1) * N], pu[:], ci)
                            ci += 1
                    hop_pend.append((ab, ua))
                for ab_, ua_ in hop_pend:
                    hop1_eng.dma_start(
                        T_d[:, :, IB * ab_:IB * (ab_ + 1), :], ua_[:])

                def hop2(c0, tg):
                    vg = vpool.tile([128, tg, R], dt_in, name=f"v{tg}")
                    src = bass.AP(T_d, c0 * 128 * R,
                                  [[R, 128], [128 * R, tg], [1, R]])
                    hop2_eng.dma_start(vg[:], src)
                    return vg

                def stageB(c0, tg, vg, ci):
                    for cb in range(tg // OB):
                        yb = ypool.tile([128, OB, R], dt_out)
                        for j in range(OB):
                            cj = cb * OB + j
                            for s in range(NSLICE):
                                py = psB.tile([128, N], f32)
                                nc.tensor.matmul(py[:], B2_sb[:],
                                                 vg[:, cj, s * N:(s + 1) * N],
                                                 start=True, stop=True)
                                copy(yb[:, j, s * N:(s + 1) * N], py[:], ci)
                                ci += 1
                        cc = c0 + cb * OB
                        out_eng.dma_start(
                            Y_d[cc:cc + OB].transpose([1, 0, 2]), yb[:])
                    return ci

                assert sum(sched) == 32
                c0 = 0
                pend = None
                for tg in sched:
                    vg = hop2(c0, tg)
                    if pend is not None:
                        ci = stageB(*pend, ci)
                    pend = (c0, tg, vg)
                    c0 += tg
                ci = stageB(*pend, ci)

            sched = cfg.get("sched")
            run = (lambda: body_sched(sched)) if sched else body
            if loop:
                with tc.For_i(0, loop, 1):
                    run()
            else:
                run()

    nc.compile()
    _NC_CACHE[key] = nc
    return nc


def _prep_inputs(x, H, mode):
    np_in = ml_dtypes.bfloat16 if mode == "bf16" else np.float32
    H64 = (np.asarray(H, dtype=np.float32)[::64, ::64] * 8.0).astype(np.float32)
    B1, B2 = _build_weights(H64)
    B1 = B1.astype(np_in)
    B2 = B2.astype(np_in)
    xf = np.asarray(x, dtype=np.float32).reshape(R_TOTAL, DIM)
    in_maps = []
    for i in range(N_CORES):
        shard = xf[i * R:(i + 1) * R]                     # (R, DIM)
        xt = np.ascontiguousarray(shard.T, dtype=np_in)   # (DIM, R)
        xt = xt.reshape(32, 128, R)
        in_maps.append({"xt": xt, "B1": B1, "B2": B2})
    return in_maps


def _unscramble(results):
    outs = []
    for i in range(N_CORES):
        Y = results[i]["Y"]                               # (32, 128, R)
        y = Y.reshape(32, 64, 2, R).transpose(3, 1, 0, 2).reshape(R, DIM)
        outs.append(y)
    return np.concatenate(outs, axis=0).reshape(4, 4096, DIM).astype(np.float32)


def _install_ntff_hook():
    """Provide antenv.axon_hooks.get_axon_ntff_profile_hook via ctypes on
    the baked libaxon_pjrt.so (the agent image lacks the module). Only used
    on the _trace path."""
    import sys, types, ctypes, contextlib
    if "antenv.axon_hooks" in sys.modules:
        return
    try:
        lib = ctypes.CDLL("/opt/axon/libaxon_pjrt.so")
        if not hasattr(lib, "axon_start_nrt_profile"):
            return
    except OSError:
        return
    lib.axon_start_nrt_profile.argtypes = [ctypes.POINTER(ctypes.c_int64),
                                           ctypes.c_size_t]
    lib.axon_start_nrt_profile.restype = ctypes.c_int64
    lib.axon_stop_nrt_profile.argtypes = [ctypes.c_char_p]
    lib.axon_stop_nrt_profile.restype = ctypes.c_int64

    @contextlib.contextmanager
    def _hook(output_dir, device_ids):
        import jax
        jax.devices()
        if device_ids:
            ids = (ctypes.c_int64 * len(device_ids))(*device_ids)
            rc = lib.axon_start_nrt_profile(ids, len(device_ids))
        else:
            rc = lib.axon_start_nrt_profile(None, 0)
        if rc != 0:
            raise RuntimeError(f"axon_start_nrt_profile rc={rc}")
        try:
            yield
        finally:
            n = lib.axon_stop_nrt_profile(str(output_dir).encode())
            print(f"ntff profile: {n} file(s) -> {output_dir}")

    mod = types.ModuleType("antenv.axon_hooks")
    mod.get_axon_ntff_profile_hook = lambda: _hook
    sys.modules["antenv.axon_hooks"] = mod


def kernel(x, H, _trace=False, _loop=0, _cfg=None):
    if _trace:
        _install_ntff_hook()
        from concourse import bass_utils as _bu
        _bu.upload_artifacts = lambda d: d
    nc = _build_bass(MODE, loop=_loop, cfg=_cfg)
    in_maps = _prep_inputs(x, H, MODE)
    res = run_bass_kernel_spmd(nc, in_maps, core_ids=list(range(N_CORES)),
                               trace=_trace)
    out = _unscramble(res.results)
    if _trace:
        return out, res
    return out
